# revision 8
# baseline (speedup 1.0000x reference)
"""Trainium2 Bass kernel for nn_Attention_68882685494025 (BEiT-style windowed
attention with relative position bias).

Sharding: data-parallel over batch (B=64 -> 8 cores x 8 batches), no
collectives. Per core, batches run in 4 pairs (394 tokens) through a fused
pipeline (one TileContext, static emission = PE program order), fully
software-pipelined at unit granularity.

v2 (this version): the qkv projections run as error-compensated fp8e4m3
DoubleRow matmuls -- contract 256/instr at 0.5 cyc/row = 4x the bf16 rate.
Weights always carry a hi+lo fp8 pair (pre-scaled x64 host-side so both
land in fp8's normal range; the 1/64 is folded into the evacuations), which
alone reproduces ~bf16 weight precision at 0.5x bf16 PE cost. The v path
adds the W_hi*x_lo correction term (3 terms, 0.75x) because v's error
passes 1:1 to the output; q/k drop it (2 terms, 0.5x) since logit noise is
tolerated by the softmax -- measured rel err 1.59e-2 vs the 2e-2 gate
(all-3-term variant: 4.0e-3, ~3.5us slower; K_Q2TERM/K_K2TERM knobs).

Layouts for the hw dual-fp8 LDWEIGHTS checks (s3_lw_dual_fp8_restrictions):
x8/x8lo are [128, cp, pl, batch, 256] with 16-aligned plane steps/offsets;
the 256 token pad also makes 512B-contiguous DMA runs, dodging the <512B
half-bandwidth DMA penalty. Weights are packed per-c-chunk [128, cp, hl,
pl, out] and DMA'd c-chunk-major to match the cp-major matmul chains
(DMA_ENGINES is serial; arrival order must track consumption order).

Stages (per pair):
  A-v) v natural [t, j] bf16 via DoubleRow (x8/x8lo c-pair planes as
       stationary, wv8 hi/lo as moving), DVE evac with 1/64.
  A-qk) q,k produced transposed [j, t] (stationary wqk8 hi/lo c-pair
       planes, moving x8, PSUM-accumulated, ACT evac: scale 1/64 +
       per-partition q/k bias, bf16).
  B)   scores transposed S.T[m, n] = kT.T @ qT per (batch, head) in bf16;
       the 8*rel-pos-bias table pre-accumulated into the same PSUM bank by
       one fp8 DoubleRow matmul (identity plane selects the head's half of
       a packed two-head fp8 table); both heads of a pair share one 2-bank
       PSUM tile; ONE fused ACT exp (scale=0.125) -> et2[m, hi, n] bf16.
  C)   O.T(+den) = [v | 1].T @ E accumulates both m-tiles. Each head's PV
       stationary carries 64 ones-columns, so the matmul lands O.T on
       partitions 0:64 and the softmax denominator REPLICATED on 64:128 for
       free (matmul cost depends only on output free size); one DVE
       reciprocal [64, TP] then feeds the DVE normalize multiplies directly
       (no gpsimd partition_broadcast -- Pool only runs the x DMAs). The
       ones regions live in 8 pinned v buffers memset once at prologue.
  D)   proj TRANSPOSED in bf16: yT[e, t] = sum_f pwT[f, e] @ OT[f, t],
       both batch halves into one PSUM bank per e-chunk, one ACT bias evac
       + one DMA per e-chunk. y leaves as bf16 (host casts to f32).

Schedule: software-pipelined emission order -- per pair, the 12 score
groups interleave with filler units of A(p+1) and D(p-1); a group's PV is
emitted after the next group's scores so ACT-exp and DVE-normalize
latencies are always covered by independent matmuls; the next pair's first
two scores groups cross the pair boundary. The prologue spreads the x
chunk DMAs across the gpsimd and ACT-HWDGE queues and paces the first
matmul chains (cp-major, solo first triple) against per-chunk arrivals;
the ACT exp table is preheated during the DMA wait.

Biases: q/k biases on the ACT evacuation (k bias is softmax-invariant but
kept for generality); v_bias and proj_b folded on host into pb_eff =
proj_b + proj_w @ v_bias (exact, since softmax rows sum to 1).

Cost-model exec 127.55 us/core (v1 all-bf16 was 154.1): PE busy 110.9 us
at 87% occupancy (qk-proj 23.6, v-proj 23.0, proj 23.6, scores 15.8, PV
15.8, rpb-init 7.9); DVE 81, ACT 79, DMA 31, Pool 12. The residual 16.4 us
of PE idle is the serial-DMA prologue floor (~6us), first-arrival and
drain-cascade bookends (~8us), and pair-end chain latencies (~2us).
"""

import os
import sys

sys.path.insert(0, "/opt/trn_rl_repo")

import numpy as np
import ml_dtypes

import concourse.bass as bass
import concourse.mybir as mybir
import concourse.tile as tile
from concourse import bacc
from concourse.bass_utils import run_bass_kernel_spmd

dt = mybir.dt
AF = mybir.ActivationFunctionType
ALU = mybir.AluOpType
PM = mybir.MatmulPerfMode

WH, WW = 14, 14
H = 12
D = 64
N = WH * WW + 1            # 197
C = 768
B_FULL = 64
N_CORES = 8
B_SH = B_FULL // N_CORES   # 8 batches per core
T = B_SH * N               # 1576 tokens per core
NPAIR = 4                  # pairs of batches per core
TP = 2 * N                 # 394 tokens per pair
NUM_REL = (2 * WH - 1) * (2 * WW - 1) + 3

WSCALE = 64.0              # fp8 weight pre-scale (host)
NPAD = 256                 # per-batch padded token stride for x8: 2 batches
                           # x 256 fp8 = 512B contiguous runs, dodging the
                           # <512B half-bandwidth DMA penalty (and 16-aligned
                           # for the dual-fp8 LDWEIGHTS checks)

# m (key) tiles of one batch
M_TILES = [(0, 128), (128, 69)]


def _gen_rel_pos_index(wh, ww):
    area = wh * ww
    coords = np.stack(np.meshgrid(np.arange(wh), np.arange(ww), indexing="ij"))
    cf = coords.reshape(2, -1)
    rel = cf[:, :, None] - cf[:, None, :]
    rel = rel.transpose(1, 2, 0).copy()
    rel[..., 0] += wh - 1
    rel[..., 1] += ww - 1
    rel[..., 0] *= 2 * ww - 1
    nrd = (2 * wh - 1) * (2 * ww - 1) + 3
    idx = np.zeros((area + 1, area + 1), dtype=np.int64)
    idx[1:, 1:] = rel.sum(-1)
    idx[0, 0:] = nrd - 3
    idx[0:, 0] = nrd - 2
    idx[0, 0] = nrd - 1
    return idx


REL_IDX = _gen_rel_pos_index(WH, WW)  # (197, 197)

# tuning knobs (env-overridable for sweeps)
_PSA_BUFS = int(os.environ.get("K_PSA_BUFS", "4"))
_PSSO_BUFS = int(os.environ.get("K_PSSO_BUFS", "2"))
_EBUFS = int(os.environ.get("K_EBUFS", "6"))
_YBUFS = int(os.environ.get("K_YBUFS", "6"))
_XTBUFS = int(os.environ.get("K_XTBUFS", "8"))
_QKTBUFS = int(os.environ.get("K_QKTBUFS", "2"))
_OTBUFS = int(os.environ.get("K_OTBUFS", "2"))
# 1 = drop the W_hi*x_lo correction on the k-side projection (k feeds only
# the softmax logits, where the extra ~7e-3 rel err is tolerated; q keeps
# 3 terms). Saves ~5.9us of PE.
_K2TERM = bool(int(os.environ.get("K_K2TERM", "1")))
# 1 = also drop the W_hi*x_lo correction on the q side (total rel err
# ~1.6e-2, still under the 2e-2 gate; saves another ~5.9us of PE)
_Q2TERM = bool(int(os.environ.get("K_Q2TERM", "1")))
# 1 = proj in 3-term fp8 DoubleRow (OT split into fp8 hi/lo after each
# head-pair column completes). Saves ~5.9us of PE but costs more in
# split-op latency/occupancy than it saves -- off by default.
_P3 = bool(int(os.environ.get("K_P3", "0")))
OSCALE = 16.0 if _P3 else 1.0

_PACE = {int(k): int(v) for k, v in
         (kv.split(':') for kv in os.environ.get('K_PACE', '').split(',') if kv)}

_CACHED = None


def _build():
    nc = bacc.Bacc(None)

    # token dim padded to NPAD per batch: the hw dual-fp8 LDWEIGHTS check
    # (s3_lw_dual_fp8_restrictions) requires 16-aligned plane steps/offsets
    # for the v-proj stationary slices
    x8_d = nc.dram_tensor(
        "x8", [128, 3, 2, B_SH, NPAD], dt.float8e4, kind="ExternalInput"
    )
    x8l_d = nc.dram_tensor(
        "x8l", [128, 3, 2, B_SH, NPAD], dt.float8e4, kind="ExternalInput"
    )
    wqk8_d = nc.dram_tensor(
        "wqk8", [128, 3, 2, 2, 2 * C], dt.float8e4, kind="ExternalInput"
    )
    wv8_d = nc.dram_tensor(
        "wv8", [128, 3, 2, 2, C], dt.float8e4, kind="ExternalInput"
    )
    if _P3:
        pwh_d = nc.dram_tensor("pwh", [128, 3, 2, C], dt.float8e4, kind="ExternalInput")
        pwl_d = nc.dram_tensor("pwl", [128, 3, 2, C], dt.float8e4, kind="ExternalInput")
    else:
        pw_d = nc.dram_tensor("pw", [128, 6, C], dt.bfloat16, kind="ExternalInput")
    rpb_d = nc.dram_tensor(
        "rpbq", [128, 6, 2, 2, N], dt.float8e4, kind="ExternalInput"
    )
    idf_d = nc.dram_tensor("idf", [128, 2, 2, 128], dt.float8e4, kind="ExternalInput")
    qkb_d = nc.dram_tensor("qkb", [128, 12], dt.float32, kind="ExternalInput")
    pbet_d = nc.dram_tensor("pbet", [128, 6], dt.float32, kind="ExternalInput")
    # y leaves the device as bf16 (the host casts back to f32): halves the
    # output DMA volume for ~5e-4 extra rel err
    y_d = nc.dram_tensor("y_sh", [128, 6, T], dt.bfloat16, kind="ExternalOutput")

    with tile.TileContext(nc) as tc:
        with (
            tc.tile_pool(name="const", bufs=1) as constp,
            tc.tile_pool(name="xt", bufs=_XTBUFS) as xtp,
            tc.tile_pool(name="qkt", bufs=_QKTBUFS) as qktp,
            tc.tile_pool(name="ep", bufs=_EBUFS) as ep,
            tc.tile_pool(name="otp", bufs=_OTBUFS) as otp,
            tc.tile_pool(name="yp", bufs=_YBUFS) as yp,
            tc.tile_pool(name="rrp", bufs=4) as rrp,
            tc.tile_pool(name="psA", bufs=_PSA_BUFS, space="PSUM") as psA,
            tc.tile_pool(name="psSO", bufs=_PSSO_BUFS, space="PSUM") as psSO,
        ):
            xts = {}

            def preload(pair, spread=False):
                """DMA one pair's x8/x8lo [p, cp, pl, t]. Steady state goes on
                the Pool SWDGE queue; the prologue pair spreads chunks across
                the gpsimd/vector/scalar SWDGE queues (those engines are idle
                at t=0) so all six generations run in parallel."""
                b0 = pair * 2
                x8 = xtp.tile([128, 3, 2, 2, NPAD], dt.float8e4, tag="xt", name=f"x8_{pair}")
                x8l = xtp.tile([128, 3, 2, 2, NPAD], dt.float8e4, tag="xt", name=f"x8l_{pair}")
                qs = (
                    [nc.gpsimd, nc.scalar, nc.gpsimd]
                    if spread
                    else [nc.gpsimd, nc.gpsimd, nc.gpsimd]
                )
                if spread:
                    # lone first half-chunk (batch 0 of cp0) for the earliest
                    # possible first matmul, then the rest in two transfers;
                    # x8l rides the ACT HWDGE queue in parallel
                    nc.gpsimd.dma_start(
                        x8[:, 0, :, 0, :], x8_d[:, 0, :, b0 : b0 + 1, :]
                    )
                    nc.gpsimd.dma_start(
                        x8[:, 0, :, 1, :], x8_d[:, 0, :, b0 + 1 : b0 + 2, :]
                    )
                    nc.gpsimd.dma_start(
                        x8[:, 1, :, :, :], x8_d[:, 1, :, b0 : b0 + 2, :]
                    )
                    nc.gpsimd.dma_start(
                        x8[:, 2, :, :, :], x8_d[:, 2, :, b0 : b0 + 2, :]
                    )
                    for cp in range(3):
                        nc.scalar.dma_start(
                            x8l[:, cp, :, :, :], x8l_d[:, cp, :, b0 : b0 + 2, :]
                        )
                else:
                    nc.gpsimd.dma_start(x8[:], x8_d[:, :, :, b0 : b0 + 2, :])
                    nc.gpsimd.dma_start(x8l[:], x8l_d[:, :, :, b0 : b0 + 2, :])
                xts[pair] = (x8, x8l)

            # PE p-state warmup: dependency-free DoubleRow matmuls on
            # memset tiles keep the PE "continuously executing" through the
            # DMA-wait prologue, so real work starts past the 3us ramp at
            # the full 2.4GHz clock instead of the 1.2GHz mid p-state
            nwarm = int(os.environ.get("K_NWARM", "0"))
            if nwarm:
                wwarm = constp.tile([128, 2, 128], dt.float8e4, name="wwarm")
                xwarm = constp.tile([128, 2, 512], dt.float8e4, name="xwarm")
                nc.vector.memset(wwarm[:], 0.0)
                nc.vector.memset(xwarm[:], 0.0)
                pswarm = psA.tile([128, 512], dt.float32, tag="big", name="pswarm")
                for _ in range(nwarm):
                    nc.tensor.matmul(
                        pswarm[:], wwarm[:], xwarm[:], start=True, stop=True,
                        perf_mode=PM.DoubleRow, skip_group_check=True,
                    )

            # ---- weights / consts on the SP queue, ordered by first use:
            # per-c-chunk packed hi+lo tiles, DMA'd c-chunk-major to match
            # the cp-major matmul chains (DMA_ENGINES is serial; arrival
            # order must track consumption order) ----
            preload(0, spread=True)
            wv8 = constp.tile([128, 3, 2, 2, C], dt.float8e4, name="wv8")
            # c0's hi half first: it alone gates the first matmul
            nc.sync.dma_start(wv8[:, 0, 0], wv8_d[:, 0, 0])
            nc.sync.dma_start(wv8[:, 0, 1], wv8_d[:, 0, 1])
            wqk8 = constp.tile([128, 3, 2, 2, 2 * C], dt.float8e4, name="wqk8")
            nc.sync.dma_start(wv8[:, 1], wv8_d[:, 1])
            nc.sync.dma_start(wqk8[:, 0], wqk8_d[:, 0])
            nc.sync.dma_start(wv8[:, 2], wv8_d[:, 2])
            nc.sync.dma_start(wqk8[:, 1], wqk8_d[:, 1])
            nc.sync.dma_start(wqk8[:, 2], wqk8_d[:, 2])
            wvh = [wv8[:, cp, 0] for cp in range(3)]
            wvl = [wv8[:, cp, 1] for cp in range(3)]
            wqkh = [wqk8[:, cp, 0] for cp in range(3)]
            wqkl = [wqk8[:, cp, 1] for cp in range(3)]
            qkb = constp.tile([128, 12], dt.float32)
            nc.sync.dma_start(qkb[:], qkb_d[:])
            # preheat the ACT exp table during the DMA-wait prologue so the
            # first scores group's exp doesn't pay the table load
            _warm = constp.tile([128, 12], dt.bfloat16, name="warm")
            nc.scalar.activation(_warm[:], qkb[:], AF.Exp, bias=0.0, scale=0.125)
            rpb = constp.tile([128, 6, 2, 2, N], dt.float8e4)
            nc.sync.dma_start(rpb[:], rpb_d[:])
            idf = constp.tile([128, 2, 2, 128], dt.float8e4)
            nc.sync.dma_start(idf[:], idf_d[:])
            if _P3:
                pwh = [constp.tile([128, 2, C], dt.float8e4, name=f"pwh{c}") for c in range(3)]
                pwl = [constp.tile([128, 2, C], dt.float8e4, name=f"pwl{c}") for c in range(3)]
                for fp in range(3):
                    nc.sync.dma_start(pwh[fp][:], pwh_d[:, fp, :, :])
                for fp in range(3):
                    nc.sync.dma_start(pwl[fp][:], pwl_d[:, fp, :, :])
            else:
                pw6 = constp.tile([128, 6, C], dt.bfloat16, name="pw6")
                nc.sync.dma_start(pw6[:], pw_d[:])
                pw = [pw6[:, c] for c in range(6)]
            pbet = constp.tile([128, 6], dt.float32)
            nc.sync.dma_start(pbet[:], pbet_d[:])

            # 8 pinned v buffers [128, H, 128]: cols 0:64 = v (rewritten per
            # pair), cols 64:128 = ones (memset once; the PV matmul then
            # replicates each head's softmax denominator onto partitions
            # 64:128 for free).
            vbufs = [
                constp.tile([128, H, 128], dt.bfloat16, name=f"vb{i}")
                for i in range(8)
            ]


            # per-pair live state
            qkts = {}  # pair -> qkT tile
            ots = {}   # pair -> OT tile
            scs = {}   # (pair, g) -> et2 tile

            def _vt(pair, bi, mt):
                return vbufs[(pair % 2) * 4 + bi * 2 + mt]

            def unit_av(pair, bi, mt, eh, interleave=None):
                """v-projection unit(s): v[t-tile, 384-half] via 9 DoubleRow
                matmuls (3 comp terms x 3 c-pairs, term-major) + DVE evac
                with 1/64."""
                x8, x8l = xts[pair]
                triples = [(bi, mt, eh)] + list(interleave or [])
                pvs = []
                for i, (tbi, tmt, teh) in enumerate(triples):
                    pvs.append(
                        psA.tile([128, 384], dt.float32, tag="big", name=f"pv{i}")
                    )
                mm = 0
                for cp in range(3):
                    for xs, wm in ((x8, wvh), (x8l, wvh), (x8, wvl)):
                        for i, (tbi, tmt, teh) in enumerate(triples):
                            m0, mn = M_TILES[tmt]
                            nc.tensor.matmul(
                                pvs[i][0:mn, :],
                                xs[:, cp, :, tbi, m0 : m0 + mn],
                                wm[cp][:, :, teh * 384 : (teh + 1) * 384],
                                start=(mm == 0),
                                stop=(mm == 8),
                                perf_mode=PM.DoubleRow,
                            )
                        mm += 1
                for i, (tbi, tmt, teh) in enumerate(triples):
                    m0, mn = M_TILES[tmt]
                    vt = _vt(pair, tbi, tmt)
                    nc.vector.tensor_scalar_mul(
                        vt[0:mn, teh * 6 : (teh + 1) * 6, 0:64],
                        pvs[i][0:mn, :].rearrange("p (h d) -> p h d", d=64),
                        1.0 / WSCALE,
                    )

            def unit_aqk(pair, j):
                """one q/k-projection unit: qkT[j, :] via 9 DoubleRow matmuls
                (term-major) + ACT bias evac with scale 1/64."""
                x8, x8l = xts[pair]
                if pair not in qkts:
                    qkts[pair] = qktp.tile([128, 12, TP], dt.bfloat16, tag="qkt", name=f"qkt{pair}")
                qkT = qkts[pair]
                pa = psA.tile([128, TP], dt.float32, tag="big")
                terms = ((wqkh, x8), (wqkh, x8l), (wqkl, x8))
                if (_K2TERM and j >= 6) or (_Q2TERM and j < 6):
                    terms = ((wqkh, x8), (wqkl, x8))
                last = 3 * len(terms) - 1
                mm = 0
                for cp in range(3):
                    for ws, xm in terms:
                        nc.tensor.matmul(
                            pa[:],
                            ws[cp][:, :, j * 128 : (j + 1) * 128],
                            xm[:, cp, :, :, 0:N],
                            start=(mm == 0),
                            stop=(mm == last),
                            perf_mode=PM.DoubleRow,
                        )
                        mm += 1
                nc.scalar.activation(
                    qkT[:, j, :], pa[:], AF.Identity,
                    bias=qkb[:, j : j + 1], scale=1.0 / WSCALE,
                )

            def unit_scores(pair, g):
                """scores for one (bi, head-pair) group: both heads into one
                (idempotent: skips if this group is already pending)
                2-bank PSUM tile (bank-aligned 512-col halves), one fused exp
                over a strided AP -> et2[p, hi, t] bf16."""
                if (pair, g) in scs:
                    return
                bi, hp = g // 6, g % 6
                qkT = qkts[pair]
                ps2 = psSO.tile(
                    [128, 2, 512], dt.float32, tag="s2", name=f"s{pair}_{g}"
                )
                for hi, h in enumerate((2 * hp, 2 * hp + 1)):
                    jq = h // 2
                    jk = 6 + h // 2
                    po = (h % 2) * 64
                    nc.tensor.matmul(
                        ps2[0:128, hi, 0:TP],
                        idf[0:128, h % 2, :, :],
                        rpb[0:128, h // 2, :, :, :],
                        start=True,
                        stop=False,
                        perf_mode=PM.DoubleRow,
                        skip_group_check=True,
                    )
                    for mt, (m0, mn) in enumerate(M_TILES):
                        nc.tensor.matmul(
                            ps2[0:mn, hi, mt * N : mt * N + N],
                            qkT[po : po + 64, jk, bi * N + m0 : bi * N + m0 + mn],
                            qkT[po : po + 64, jq, bi * N : (bi + 1) * N],
                            start=False,
                            stop=(mt == 1),
                            skip_group_check=True,
                        )
                et2 = ep.tile([128, 2, TP], dt.bfloat16, tag="et", name=f"et{pair}_{g}")
                nc.scalar.activation(
                    et2[:], ps2[:, :, 0:TP], AF.Exp, bias=0.0, scale=0.125
                )
                scs[(pair, g)] = et2

            def unit_pv(pair, g):
                """PV + normalization for one group -> OT[f, t] bf16. The PV
                stationary's ones-columns land the denominator replicated on
                partitions 64:128; one DVE reciprocal then feeds the
                normalize multiplies directly."""
                bi, hp = g // 6, g % 6
                et2 = scs.pop((pair, g))
                if pair not in ots:
                    OT_ = otp.tile([128, 6, TP], dt.bfloat16, tag="ot", name=f"ot{pair}")
                    if _P3:
                        O8h_ = otp.tile([128, 6, TP], dt.float8e4, tag="o8h", name=f"o8h{pair}")
                        O8l_ = otp.tile([128, 6, TP], dt.float8e4, tag="o8l", name=f"o8l{pair}")
                        ots[pair] = (OT_, O8h_, O8l_)
                    else:
                        ots[pair] = (OT_, None, None)
                OT = ots[pair][0]
                po_t = psA.tile([128, TP], dt.float32, tag="big", name=f"o{pair}_{g}")
                for hi, h in enumerate((2 * hp, 2 * hp + 1)):
                    nc.tensor.matmul(
                        po_t[:, hi * N : hi * N + N],
                        _vt(pair, bi, 0)[:, h, :],
                        et2[0:128, hi, 0:N],
                        start=True,
                        stop=False,
                    )
                    nc.tensor.matmul(
                        po_t[:, hi * N : hi * N + N],
                        _vt(pair, bi, 1)[0:69, h, :],
                        et2[0:69, hi, N : 2 * N],
                        start=False,
                        stop=True,
                    )
                rb = rrp.tile([64, TP], dt.float32, tag="rb")
                nc.vector.reciprocal(rb[:], po_t[64:128, :])
                for hi, h in enumerate((2 * hp, 2 * hp + 1)):
                    nc.vector.tensor_tensor(
                        OT[
                            (h % 2) * 64 : (h % 2) * 64 + 64,
                            h // 2,
                            bi * N : (bi + 1) * N,
                        ],
                        po_t[0:64, hi * N : hi * N + N],
                        rb[:, hi * N : hi * N + N],
                        ALU.mult,
                    )

            def unit_otsplit(pair, fp):
                """fp8 hi/lo split of one f-chunk-pair strip of the pair's
                (16x-scaled) OT, once both its head columns are normalized:
                one ACT cast + one Pool subtract."""
                OT, O8h, O8l = ots[pair]
                nc.gpsimd.tensor_copy(
                    O8h[:, 2 * fp : 2 * fp + 2, :], OT[:, 2 * fp : 2 * fp + 2, :]
                )
                nc.vector.tensor_sub(
                    O8l[:, 2 * fp : 2 * fp + 2, :],
                    OT[:, 2 * fp : 2 * fp + 2, :],
                    O8h[:, 2 * fp : 2 * fp + 2, :],
                )

            pds = {}  # (pair, ec) -> shared pd2 PSUM tile
            yts = {}  # (pair, ec) -> yt tile (last pair only)

            def unit_d(pair, bi, ec):
                """one proj unit: yT[e-chunk, batch-half]. Both batch halves
                share one PSUM bank; one ACT bias evac + one DMA per e-chunk
                once the second half lands."""
                OT, O8h, O8l = ots[pair]
                if (pair, ec) not in pds:
                    pds[(pair, ec)] = psA.tile(
                        [128, TP], dt.float32, tag="big", name=f"pd{pair}_{ec}"
                    )
                pd2 = pds[(pair, ec)]
                if _P3:
                    mm = 0
                    for ws, om in ((pwh, O8h), (pwh, O8l), (pwl, O8h)):
                        for fp in range(3):
                            nc.tensor.matmul(
                                pd2[:, bi * N : (bi + 1) * N],
                                ws[fp][:, :, ec * 128 : (ec + 1) * 128],
                                om[:, 2 * fp : 2 * fp + 2, bi * N : (bi + 1) * N],
                                start=(mm == 0),
                                stop=(mm == 8),
                                perf_mode=PM.DoubleRow,
                                skip_group_check=True,
                            )
                            mm += 1
                else:
                    for f in range(6):
                        nc.tensor.matmul(
                            pd2[:, bi * N : (bi + 1) * N],
                            pw[f][:, ec * 128 : (ec + 1) * 128],
                            OT[:, f, bi * N : (bi + 1) * N],
                            start=(f == 0),
                            stop=(f == 5),
                            skip_group_check=True,
                        )
                t0 = pair * TP
                if pair == NPAIR - 1 and ec == 5:
                    # very last e-chunk: evacuate + DMA each batch half
                    # separately so the final transfer after the last matmul
                    # is halved
                    if (pair, ec) not in yts:
                        yts[(pair, ec)] = yp.tile(
                            [128, TP], dt.bfloat16, tag="yt", name=f"yt{pair}_{ec}"
                        )
                    yt = yts[(pair, ec)]
                    nc.scalar.activation(
                        yt[:, bi * N : (bi + 1) * N], pd2[:, bi * N : (bi + 1) * N],
                        AF.Identity, bias=pbet[:, ec : ec + 1],
                        scale=1.0 / (WSCALE * OSCALE) if _P3 else 1.0,
                    )
                    nc.sync.dma_start(
                        y_d[:, ec, t0 + bi * N : t0 + (bi + 1) * N],
                        yt[:, bi * N : (bi + 1) * N],
                    )
                    if bi == 1:
                        del pds[(pair, ec)]
                        del yts[(pair, ec)]
                elif bi == 1:
                    yt = yp.tile([128, TP], dt.bfloat16, tag="yt", name=f"yt{pair}_{ec}")
                    nc.scalar.activation(
                        yt[:], pd2[:], AF.Identity, bias=pbet[:, ec : ec + 1],
                        scale=1.0 / (WSCALE * OSCALE) if _P3 else 1.0,
                    )
                    nc.sync.dma_start(y_d[:, ec, t0 : t0 + TP], yt[:])
                    del pds[(pair, ec)]

            AQK_ORDER = [0, 6, 1, 7, 2, 8, 3, 9, 4, 10, 5, 11]

            def a_units(pair, prologue=False):
                avs = [
                    (bi, mt, eh) for bi in range(2) for mt in range(2) for eh in range(2)
                ]
                if prologue:
                    # solo first triple (smallest dependency set: x8[cp0,b0]
                    # + wvh arrive first), then a 3-triple interleave riding
                    # the remaining chunk arrivals
                    yield lambda: unit_av(pair, *avs[0])
                    yield lambda: unit_av(pair, *avs[1], interleave=avs[2:4])
                    avs = avs[4:]
                for bi, mt, eh in avs:
                    yield lambda bi=bi, mt=mt, eh=eh: unit_av(pair, bi, mt, eh)
                for j in AQK_ORDER:
                    yield lambda j=j: unit_aqk(pair, j)

            def d_units(pair, binner=True):
                if binner:
                    for ec in range(6):
                        for bi in range(2):
                            yield lambda bi=bi, ec=ec: unit_d(pair, bi, ec)
                else:
                    for bi in range(2):
                        for ec in range(6):
                            yield lambda bi=bi, ec=ec: unit_d(pair, bi, ec)

            # prologue: pair 0's qkv stage straight up
            for u in a_units(0, prologue=True):
                u()

            # steady state: per pair, scores(g+1) and filler units (A units,
            # D(p-1)) are emitted between scores(g)'s exp and PV(g).
            plans = {
                0: lambda: list(a_units(1)),
                1: lambda: list(a_units(2)) + list(d_units(0)),
                2: lambda: list(a_units(3)) + list(d_units(1)),
                3: lambda: list(d_units(2)),
            }
            for pair in range(NPAIR):
                fillers = plans[pair]()
                nf = len(fillers)
                done = 0
                if pair == 0:
                    # ones-region memsets deferred past the prologue so the
                    # context-entry barrier doesn't hold the first matmuls
                    # (only PVs read them, from ~21us on)
                    for i in range(8):
                        nc.vector.memset(vbufs[i][:, :, 64:128], 1.0 / OSCALE)
                    unit_scores(pair, 0)
                    # pair 1's x arrives during BC(0): deferred past the
                    # prologue consts so its transfers don't displace wqk8
                    preload(1)
                for g in range(12):
                    want = ((g + 1) * nf + _PACE.get(pair, 6)) // 12
                    while done < want:
                        fillers[done]()
                        done += 1
                    if g + 1 < 12:
                        unit_scores(pair, g + 1)
                    elif pair + 1 < NPAIR:
                        unit_scores(pair + 1, 0)
                        unit_scores(pair + 1, 1)
                    unit_pv(pair, g)
                    if _P3 and g in (7, 9, 11):
                        unit_otsplit(pair, (g - 7) // 2)
                    if g == 0 and pair + 2 < NPAIR:
                        preload(pair + 2)
                qkts.pop(pair, None)
            for u in d_units(NPAIR - 1):
                u()

    nc.finalize()
    return nc


def _host_prep(x, qkv_w, q_bias, k_bias, v_bias, rel_table, proj_w, proj_b):
    f32 = np.float32
    bf16 = ml_dtypes.bfloat16
    fp8 = ml_dtypes.float8_e4m3

    # x: [B, N, C] f32 -> per-core fp8 hi/lo [128, 3, 2, B_SH, NPAD]
    # (c = cp*256 + pl*128 + p; token dim padded to NPAD per batch)
    x_t = x.reshape(N_CORES, B_SH, N, 3, 2, 128).transpose(0, 5, 3, 4, 1, 2)
    # [8, 128, 3, 2, B_SH, N] f32
    x_p = np.zeros(x_t.shape[:-1] + (NPAD,), dtype=f32)
    x_p[..., :N] = x_t
    x8 = x_p.astype(fp8)
    x8l = (x_p - x8.astype(f32)).astype(fp8)

    def wsplit(wT):
        # wT: [c, out] f32 -> packed hi/lo fp8 [128, cp, hl, pl, out]
        # (scaled x64)
        ws = WSCALE * wT
        hi = ws.astype(fp8)
        lo = (ws - hi.astype(f32)).astype(fp8)
        def lay(a):
            return a.reshape(3, 2, 128, wT.shape[1]).transpose(2, 0, 1, 3)
        return np.ascontiguousarray(
            np.stack([lay(hi), lay(lo)], axis=2)
        )  # [128, 3, 2, 2, out]

    wqk8 = wsplit(np.ascontiguousarray(qkv_w[: 2 * C].T))
    wv8 = wsplit(np.ascontiguousarray(qkv_w[2 * C :].T))

    if _P3:
        raise NotImplementedError("P3 host prep not updated for packed wsplit")
    else:
        pw_T = np.ascontiguousarray(proj_w.T)       # [f, e]
        pw_h = np.ascontiguousarray(
            pw_T.reshape(6, 128, C).transpose(1, 0, 2)
        ).astype(bf16)

    # packed two-head fp8 rel-pos table for the DoubleRow bias matmul
    rpb_full = rel_table[REL_IDX]                   # [n, m, H]
    R8T = 8.0 * rpb_full.transpose(2, 1, 0)         # [H, m, n]
    rpb_h = np.zeros((128, 6, 2, 2, N), dtype=fp8)
    for mt, (m0, mn) in enumerate(M_TILES):
        blk = R8T[:, m0 : m0 + mn, :].astype(fp8)   # [H, mn, n]
        rpb_h[:mn, :, :, mt, :] = blk.reshape(6, 2, mn, N).transpose(2, 0, 1, 3)
    idf_h = np.zeros((128, 2, 2, 128), dtype=fp8)
    eye = np.eye(128, dtype=fp8)
    idf_h[:, 0, 0, :] = eye
    idf_h[:, 1, 1, :] = eye

    qkb_h = np.ascontiguousarray(
        np.concatenate([q_bias, k_bias]).reshape(12, 128).T
    ).astype(f32)
    pbe = (proj_b + proj_w @ v_bias).astype(f32)    # [C]
    pbet_h = np.ascontiguousarray(pbe.reshape(6, 128).T).astype(f32)  # [128, 6]

    shared = {
        "wqk8": wqk8,
        "wv8": wv8,
        "rpbq": rpb_h,
        "idf": idf_h,
        "qkb": qkb_h,
        "pbet": pbet_h,
    }
    shared["pw"] = pw_h
    return [dict(shared, x8=x8[i], x8l=x8l[i]) for i in range(N_CORES)]


def kernel(**inputs):
    global _CACHED
    if _CACHED is None:
        _CACHED = _build()
    nc = _CACHED

    in_maps = _host_prep(
        np.asarray(inputs["x"], np.float32),
        np.asarray(inputs["qkv_w"], np.float32),
        np.asarray(inputs["q_bias"], np.float32),
        np.asarray(inputs["k_bias"], np.float32),
        np.asarray(inputs["v_bias"], np.float32),
        np.asarray(inputs["rel_table"], np.float32),
        np.asarray(inputs["proj_w"], np.float32),
        np.asarray(inputs["proj_b"], np.float32),
    )

    trace = bool(int(os.environ.get("BASS_KERNEL_TRACE", "0")))
    res = run_bass_kernel_spmd(
        nc, in_maps, core_ids=list(range(N_CORES)), trace=trace
    )
    if trace and res.exec_time_ns is not None:
        print(f"HW exec time: {res.exec_time_ns} ns")
        if res.instructions_and_trace is not None:
            print(f"trace: {res.instructions_and_trace[1]}")

    y = np.stack(
        [np.asarray(r["y_sh"], dtype=np.float32) for r in res.results], axis=0
    )  # [8, 128, 6, T]
    y = y.transpose(0, 3, 2, 1).reshape(N_CORES, T, C)      # [8, T, C]
    return np.ascontiguousarray(y.reshape(B_FULL, N, C))
